# revision 4
# baseline (speedup 1.0000x reference)
"""Multi-head attention (B=4, N=2048, C=768, H=12, Dh=64) on 8 TRN2 NeuronCores.

Sharding: 2 cores per batch (data parallel on batch x sequence-split on query
rows). Each core owns 1024 query rows but computes K/V for its batch's full
2048-token sequence from host-pre-transposed activations (own half first, so
the graph is identical across cores). No collectives; the host concatenates
the 8 [1024, 768] output row-blocks.

Per-core inputs (partition dim first):
  xT     [768, 2048]  bf16  x[b].T, own-half tokens first
  wqkvT  [768, 2304]  bf16  qkv_w.T (cols 0:768 q, 768:1536 k, 1536:2304 v)
  wprojT [768, 768]   bf16  proj_w.T
  bias   [1, 768]     f32
  out    [1024, 768]  f32

v2 pipeline (from trace analysis of the 371us baseline):
  - score matmuls for the two heads of a pair are issued adjacently with lhsT
    base partitions 0/64 -> the PE row-tiles them concurrently (measured
    ~215ns per pair instead of 2x246ns serial).
  - scores land in a single [128, 2048] PSUM tile (4 banks: h0/h1 x j/j+1)
    and ONE 2048-wide exp covers them (fewer ACTIVATE instructions).
  - PV stationary is a 128-col window into the packed V tile
    [v_h|ones|v_{h+1}...] -> FWL weight loads; rows 65..127 of the PSUM
    accumulator are garbage and never read.
  - only q/k for head-pair 0 plus all of V are computed up front; q/k for
    later head pairs and the ib-0 projection matmuls are interleaved into the
    exp-bound attention steps so the PE never idles.
"""

import sys

if "/opt/trn_rl_repo" not in sys.path:
    sys.path.insert(0, "/opt/trn_rl_repo")

import numpy as np
import ml_dtypes

B, N, C = 4, 2048, 768
H, Dh = 12, 64
NQ = 1024          # query rows owned by one core
SCALE = Dh ** -0.5
CCH = C // 128     # 6 contraction chunks
NCORES = 8
VW = 11 * 65 + 128  # padded width of packed v tiles (last head's 128-window)

_NC_CACHE = {}


def _build():
    import concourse.bass as bass
    import concourse.tile as tile
    import concourse.mybir as mybir
    from concourse import bacc

    f32 = mybir.dt.float32
    bf16 = mybir.dt.bfloat16
    Exp = mybir.ActivationFunctionType.Exp

    nc = bacc.Bacc(
        "TRN2",
        target_bir_lowering=False,
        debug=False,
        enable_asserts=False,
        num_devices=NCORES,
    )

    xT = nc.dram_tensor("xT", [C, N], bf16, kind="ExternalInput").ap()
    wqkvT = nc.dram_tensor("wqkvT", [C, 3 * C], bf16, kind="ExternalInput").ap()
    wprojT = nc.dram_tensor("wprojT", [C, C], bf16, kind="ExternalInput").ap()
    bias = nc.dram_tensor("bias", [1, C], f32, kind="ExternalInput").ap()
    out = nc.dram_tensor("out", [NQ, C], f32, kind="ExternalOutput").ap()

    with tile.TileContext(nc) as tc:
        from contextlib import ExitStack

        with ExitStack() as ctx:
            singles = ctx.enter_context(tc.tile_pool(name="singles", bufs=1))
            psum = ctx.enter_context(tc.tile_pool(name="psum", bufs=1, space="PSUM"))
            work = ctx.enter_context(tc.tile_pool(name="work", bufs=4))

            # ---- input DMAs --------------------------------------------
            load = tc.alloc_tile_pool(name="load", bufs=1)
            xt = [load.tile([128, N], bf16, tag=f"xt{i}", name=f"xt{i}")
                  for i in range(CCH)]
            wq = [load.tile([128, 3 * C], bf16, tag=f"wq{i}", name=f"wq{i}")
                  for i in range(CCH)]
            # xt on the sync queue; wq slices on the gpsimd queue, ordered so
            # the first matmuls can start as early as possible:
            #   q/k cols for head-pair 0, then v cols, then the rest of q/k.
            for i in range(CCH):
                nc.sync.dma_start(out=xt[i], in_=xT[i * 128:(i + 1) * 128, :])
                nc.gpsimd.dma_start(out=wq[i][:, 0:128],
                                    in_=wqkvT[i * 128:(i + 1) * 128, 0:128])
                nc.gpsimd.dma_start(out=wq[i][:, C:C + 128],
                                    in_=wqkvT[i * 128:(i + 1) * 128, C:C + 128])
            for i in range(CCH):
                nc.gpsimd.dma_start(out=wq[i][:, 2 * C:3 * C],
                                    in_=wqkvT[i * 128:(i + 1) * 128, 2 * C:3 * C])
            for i in range(CCH):
                nc.gpsimd.dma_start(out=wq[i][:, 128:C],
                                    in_=wqkvT[i * 128:(i + 1) * 128, 128:C])
                nc.gpsimd.dma_start(out=wq[i][:, C + 128:2 * C],
                                    in_=wqkvT[i * 128:(i + 1) * 128, C + 128:2 * C])
            wp = []
            for i in range(CCH):
                t = singles.tile([128, C], bf16, tag=f"wp{i}", name=f"wp{i}")
                nc.sync.dma_start(out=t, in_=wprojT[i * 128:(i + 1) * 128, :])
                wp.append(t)
            bias_bc = singles.tile([128, C], f32, tag="bias", name="bias_bc")
            nc.sync.dma_start(
                out=bias_bc,
                in_=bass.AP(tensor=bias.tensor, offset=bias.offset,
                            ap=[[0, 128]] + list(bias.ap[1:])),
            )

            # ---- qkv storage -------------------------------------------
            qt = [singles.tile([128, NQ], bf16, tag=f"qt{i}", name=f"qt{i}")
                  for i in range(CCH)]
            kt = [singles.tile([128, N], bf16, tag=f"kt{i}", name=f"kt{i}")
                  for i in range(CCH)]
            vt = [singles.tile([128, VW], bf16, tag=f"vt{i}", name=f"vt{i}")
                  for i in range(N // 128)]
            att = [singles.tile([128, NQ], bf16, tag=f"att{i}", name=f"att{i}")
                   for i in range(CCH)]

            # q/k projection for one head pair (kept psum-resident across the
            # contraction so the stationary is reused between n-chunks).
            def emit_qk(hp):
                ops = []
                for nch in range(NQ // 512):
                    ps = psum.tile([128, 512], f32, tag="qk", bufs=2,
                                   name=f"psq{hp}{nch}")
                    for cc in range(CCH):
                        ops.append(lambda hp=hp, nch=nch, cc=cc, ps=ps: nc.tensor.matmul(
                            ps,
                            lhsT=wq[cc][:, hp * 128:(hp + 1) * 128],
                            rhs=xt[cc][:, nch * 512:(nch + 1) * 512],
                            start=(cc == 0), stop=(cc == CCH - 1),
                        ))
                    ops.append(lambda hp=hp, nch=nch, ps=ps: nc.vector.tensor_copy(
                        qt[hp][:, nch * 512:(nch + 1) * 512], ps))
                for nch in range(N // 512):
                    ps = psum.tile([128, 512], f32, tag="qk", bufs=2,
                                   name=f"psk{hp}{nch}")
                    for cc in range(CCH):
                        ops.append(lambda hp=hp, nch=nch, cc=cc, ps=ps: nc.tensor.matmul(
                            ps,
                            lhsT=wq[cc][:, C + hp * 128:C + (hp + 1) * 128],
                            rhs=xt[cc][:, nch * 512:(nch + 1) * 512],
                            start=(cc == 0), stop=(cc == CCH - 1),
                        ))
                    ops.append(lambda hp=hp, nch=nch, ps=ps: nc.vector.tensor_copy(
                        kt[hp][:, nch * 512:(nch + 1) * 512], ps))
                return ops

            # proj for one 128-row output block (12 matmuls + add + dma out)
            def emit_proj(ic):
                ops = []
                pjs = []
                for (d0, dw) in ((0, 512), (512, 256)):
                    pj = psum.tile([128, 512], f32, tag="qk", bufs=2,
                                   name=f"pj{ic}_{d0}")
                    pjs.append((pj, d0, dw))
                    for cc in range(CCH):
                        ops.append(lambda ic=ic, d0=d0, dw=dw, cc=cc, pj=pj: nc.tensor.matmul(
                            pj[:, 0:dw],
                            lhsT=att[cc][:, ic * 128:(ic + 1) * 128],
                            rhs=wp[cc][:, d0:d0 + dw],
                            start=(cc == 0), stop=(cc == CCH - 1),
                        ))
                def fin(ic=ic, pjs=pjs):
                    osb = work.tile([128, C], f32, tag="osb", bufs=3,
                                    name=f"osb{ic}")
                    for (pj, d0, dw) in pjs:
                        nc.vector.tensor_add(osb[:, d0:d0 + dw], pj[:, 0:dw],
                                             bias_bc[:, d0:d0 + dw])
                    nc.sync.dma_start(out=out[ic * 128:(ic + 1) * 128, :],
                                      in_=osb)
                ops.append(fin)
                return ops

            # ---- phase 0: q/k for head pair 0, then v ------------------
            for op in emit_qk(0):
                op()

            # v in [token, d] layout, packed [v_h(64)|1] x 12 heads + pad.
            for nt in range(N // 128):
                vaug = vt[nt][:, 0:H * 65].rearrange("p (h e) -> p h e", e=65)
                nc.vector.memset(vaug[:, :, 64:65], 1.0)
                nc.vector.memset(vt[nt][:, H * 65:VW], 0.0)
                for (d0, dw) in ((0, 512), (512, 256)):
                    ps = psum.tile([128, 512], f32, tag="qk", bufs=2,
                                   name=f"psv{nt}")
                    for cc in range(CCH):
                        nc.tensor.matmul(
                            ps[:, :dw],
                            lhsT=xt[cc][:, nt * 128:(nt + 1) * 128],
                            rhs=wq[cc][:, 2 * C + d0:2 * C + d0 + dw],
                            start=(cc == 0), stop=(cc == CCH - 1),
                        )
                    nc.vector.tensor_copy(
                        vaug[:, d0 // 64:(d0 + dw) // 64, 0:64],
                        ps[:, :dw].rearrange("p (h e) -> p h e", e=64),
                    )

            # ---- attention ---------------------------------------------
            # filler queue: matmul-ish ops to interleave into exp-bound steps
            filler = []
            for hp in range(1, CCH):
                filler.extend(emit_qk(hp))

            def drain_filler(k):
                for _ in range(k):
                    if filler:
                        filler.pop(0)()

            NJ = N // 128                     # 16 kv chunks
            for ib in range(NQ // 512):       # 512-wide query block
                for hp in range(CCH):         # head pair
                    pv = [psum.tile([128, 512], f32, tag="pv", bufs=2,
                                    name=f"pv{h2}") for h2 in range(2)]
                    for j0 in range(0, NJ, 2):   # two kv chunks per step
                        st = psum.tile([128, 2048], f32, tag="st", bufs=1,
                                       name="st")
                        for cx in range(2):
                            j = j0 + cx
                            for h2 in range(2):
                                hb = h2 * 64
                                nc.tensor.matmul(
                                    st[:, (2 * cx + h2) * 512:(2 * cx + h2 + 1) * 512],
                                    lhsT=kt[hp][hb:hb + 64, j * 128:(j + 1) * 128],
                                    rhs=qt[hp][hb:hb + 64, ib * 512:(ib + 1) * 512],
                                    start=True, stop=True,
                                )
                        et = work.tile([128, 2048], bf16, tag="et", bufs=3,
                                       name="et")
                        nc.scalar.activation(et, st, Exp, scale=SCALE)
                        drain_filler(5)
                        for cx in range(2):
                            j = j0 + cx
                            for h2 in range(2):
                                h = hp * 2 + h2
                                nc.tensor.matmul(
                                    pv[h2],
                                    lhsT=vt[j][:, h * 65:h * 65 + 128],
                                    rhs=et[:, (2 * cx + h2) * 512:(2 * cx + h2 + 1) * 512],
                                    start=(j == 0), stop=(j == NJ - 1),
                                )
                    for h2 in range(2):
                        srow = work.tile([1, 512], f32, tag="srow", bufs=4,
                                         name="srow")
                        nc.vector.tensor_copy(srow, pv[h2][64:65, :])
                        sinv = work.tile([1, 512], f32, tag="sinv", bufs=4,
                                         name="sinv")
                        nc.vector.reciprocal_approx_fast(sinv, srow)
                        bc = work.tile([64, 512], f32, tag="bc", bufs=4,
                                       name="bc")
                        nc.gpsimd.partition_broadcast(bc, sinv)
                        nc.vector.tensor_mul(
                            att[hp][h2 * 64:h2 * 64 + 64, ib * 512:(ib + 1) * 512],
                            pv[h2][0:64, :],
                            bc,
                        )
                if ib == 0:
                    # release the load pool once qkv is fully consumed
                    assert not filler
                    load.release()
                    filler = []
                    for ic in range(0, 4):
                        filler.extend(emit_proj(ic))
            # tail: proj for the second query block
            for op in filler:
                op()
            for ic in range(4, 8):
                for op in emit_proj(ic):
                    op()

    nc.compile()
    return nc


def _get_nc():
    if "nc" not in _NC_CACHE:
        _NC_CACHE["nc"] = _build()
    return _NC_CACHE["nc"]


def _ensure_ntff_hook():
    """The agent image's ``antenv`` lacks ``axon_hooks``; synthesize it so
    ``run_bass_kernel_spmd(trace=True)`` can capture NTFF profiles."""
    import types
    try:
        from antenv.axon_hooks import get_axon_ntff_profile_hook  # noqa: F401
        return
    except ImportError:
        pass
    import antenv
    from trn_agent_boot.trn_boot import _ntff_profile_via_ctypes
    hook = _ntff_profile_via_ctypes("/opt/axon/libaxon_pjrt.so")
    mod = types.ModuleType("antenv.axon_hooks")
    mod._hook = hook
    mod.get_axon_ntff_profile_hook = lambda: mod._hook

    def _set(h):
        mod._hook = h

    mod.set_axon_ntff_profile_hook = _set
    sys.modules["antenv.axon_hooks"] = mod
    antenv.axon_hooks = mod


def kernel(trace=False, **inputs):
    x = np.asarray(inputs["x"], np.float32)
    qkv_w = np.asarray(inputs["qkv_w"], np.float32)
    proj_w = np.asarray(inputs["proj_w"], np.float32)
    proj_b = np.asarray(inputs["proj_b"], np.float32)

    nc = _get_nc()

    xTb = np.ascontiguousarray(x.transpose(0, 2, 1)).astype(ml_dtypes.bfloat16)
    wqkvT = np.ascontiguousarray(qkv_w.T).astype(ml_dtypes.bfloat16)
    wprojT = np.ascontiguousarray(proj_w.T).astype(ml_dtypes.bfloat16)
    bias = np.ascontiguousarray(proj_b.reshape(1, C))

    in_maps = []
    for c in range(NCORES):
        b, half = divmod(c, 2)
        if half == 0:
            xTc = xTb[b]
        else:
            xTc = np.concatenate([xTb[b][:, NQ:], xTb[b][:, :NQ]], axis=1)
        in_maps.append({
            "xT": np.ascontiguousarray(xTc),
            "wqkvT": wqkvT,
            "wprojT": wprojT,
            "bias": bias,
        })

    from concourse import bass_utils
    if trace:
        _ensure_ntff_hook()
        bass_utils.upload_artifacts = lambda tmpdir: tmpdir
    res = bass_utils.run_bass_kernel_spmd(
        nc, in_maps, core_ids=list(range(NCORES)), trace=trace,
    )

    out = np.empty((B, N, C), np.float32)
    for c in range(NCORES):
        b, half = divmod(c, 2)
        out[b, half * NQ:(half + 1) * NQ, :] = res.results[c]["out"]

    if trace:
        return out, res
    return out


# revision 6
# speedup vs baseline: 1.5142x; 1.5142x over previous
"""Multi-head attention (B=4, N=2048, C=768, H=12, Dh=64) on 8 TRN2 NeuronCores.

Sharding: 2 cores per batch (data parallel on batch x sequence-split on query
rows). Each core owns 1024 query rows but computes K/V for its batch's full
2048-token sequence from host-pre-transposed activations (own half first, so
the graph is identical across cores). No collectives; the host concatenates
the 8 [1024, 768] output row-blocks.

Per-core inputs (partition dim first):
  xT     [768, 2048]  bf16  x[b].T, own-half tokens first
  wqkvT  [768, 2304]  bf16  qkv_w.T (cols 0:768 q, 768:1536 k, 1536:2304 v)
  wprojT [768, 768]   bf16  proj_w.T
  bias   [1, 768]     f32
  out    [1024, 768]  f32

v2 pipeline (from trace analysis of the 371us baseline):
  - score matmuls for the two heads of a pair are issued adjacently with lhsT
    base partitions 0/64 -> the PE row-tiles them concurrently (measured
    ~215ns per pair instead of 2x246ns serial).
  - scores land in a single [128, 2048] PSUM tile (4 banks: h0/h1 x j/j+1)
    and ONE 2048-wide exp covers them (fewer ACTIVATE instructions).
  - PV stationary is a 128-col window into the packed V tile
    [v_h|ones|v_{h+1}...] -> FWL weight loads; rows 65..127 of the PSUM
    accumulator are garbage and never read.
  - only q/k for head-pair 0 plus all of V are computed up front; q/k for
    later head pairs and the ib-0 projection matmuls are interleaved into the
    exp-bound attention steps so the PE never idles.
"""

import sys

if "/opt/trn_rl_repo" not in sys.path:
    sys.path.insert(0, "/opt/trn_rl_repo")

import numpy as np
import ml_dtypes

B, N, C = 4, 2048, 768
H, Dh = 12, 64
NQ = 1024          # query rows owned by one core
SCALE = Dh ** -0.5
CCH = C // 128     # 6 contraction chunks
NCORES = 8
VW = 11 * 65 + 128  # padded width of packed v tiles (last head's 128-window)

_NC_CACHE = {}


def _build():
    import concourse.bass as bass
    import concourse.tile as tile
    import concourse.mybir as mybir
    from concourse import bacc

    f32 = mybir.dt.float32
    bf16 = mybir.dt.bfloat16
    Exp = mybir.ActivationFunctionType.Exp

    nc = bacc.Bacc(
        "TRN2",
        target_bir_lowering=False,
        debug=False,
        enable_asserts=False,
        num_devices=NCORES,
    )

    xT = nc.dram_tensor("xT", [C, N], bf16, kind="ExternalInput").ap()
    wqkvT = nc.dram_tensor("wqkvT", [C, 3 * C], bf16, kind="ExternalInput").ap()
    wprojT = nc.dram_tensor("wprojT", [C, C], bf16, kind="ExternalInput").ap()
    bias = nc.dram_tensor("bias", [1, C], f32, kind="ExternalInput").ap()
    out = nc.dram_tensor("out", [NQ, C], f32, kind="ExternalOutput").ap()

    with tile.TileContext(nc) as tc:
        from contextlib import ExitStack

        with ExitStack() as ctx:
            singles = ctx.enter_context(tc.tile_pool(name="singles", bufs=1))
            psum = ctx.enter_context(tc.tile_pool(name="psum", bufs=1, space="PSUM"))
            work = ctx.enter_context(tc.tile_pool(name="work", bufs=4))

            # ---- input DMAs --------------------------------------------
            load = tc.alloc_tile_pool(name="load", bufs=1)
            xt = [load.tile([128, N], bf16, tag=f"xt{i}", name=f"xt{i}")
                  for i in range(CCH)]
            wq = [load.tile([128, 3 * C], bf16, tag=f"wq{i}", name=f"wq{i}")
                  for i in range(CCH)]
            # xt on the sync queue; wq slices on the gpsimd queue, ordered so
            # the first matmuls can start as early as possible:
            #   q/k cols for head-pair 0, then v cols, then the rest of q/k.
            for i in range(CCH):
                nc.sync.dma_start(out=xt[i], in_=xT[i * 128:(i + 1) * 128, :])
                nc.gpsimd.dma_start(out=wq[i][:, 0:128],
                                    in_=wqkvT[i * 128:(i + 1) * 128, 0:128])
                nc.gpsimd.dma_start(out=wq[i][:, C:C + 128],
                                    in_=wqkvT[i * 128:(i + 1) * 128, C:C + 128])
            for i in range(CCH):
                nc.gpsimd.dma_start(out=wq[i][:, 2 * C:3 * C],
                                    in_=wqkvT[i * 128:(i + 1) * 128, 2 * C:3 * C])
            for i in range(CCH):
                nc.gpsimd.dma_start(out=wq[i][:, 128:C],
                                    in_=wqkvT[i * 128:(i + 1) * 128, 128:C])
                nc.gpsimd.dma_start(out=wq[i][:, C + 128:2 * C],
                                    in_=wqkvT[i * 128:(i + 1) * 128, C + 128:2 * C])
            wp = []
            for i in range(CCH):
                t = singles.tile([128, C], bf16, tag=f"wp{i}", name=f"wp{i}")
                nc.sync.dma_start(out=t, in_=wprojT[i * 128:(i + 1) * 128, :])
                wp.append(t)
            bias_bc = singles.tile([128, C], f32, tag="bias", name="bias_bc")
            nc.sync.dma_start(
                out=bias_bc,
                in_=bass.AP(tensor=bias.tensor, offset=bias.offset,
                            ap=[[0, 128]] + list(bias.ap[1:])),
            )

            # ---- qkv storage -------------------------------------------
            qt = [singles.tile([128, NQ], bf16, tag=f"qt{i}", name=f"qt{i}")
                  for i in range(CCH)]
            kt = [singles.tile([128, N], bf16, tag=f"kt{i}", name=f"kt{i}")
                  for i in range(CCH)]
            vt = [singles.tile([128, VW], bf16, tag=f"vt{i}", name=f"vt{i}")
                  for i in range(N // 128)]
            att = [singles.tile([128, NQ], bf16, tag=f"att{i}", name=f"att{i}")
                   for i in range(CCH)]

            # q/k projection for one head pair (kept psum-resident across the
            # contraction so the stationary is reused between n-chunks).
            def emit_qk(hp):
                ops = []
                for nch in range(NQ // 512):
                    ps = psum.tile([128, 512], f32, tag="qk", bufs=2,
                                   name=f"psq{hp}{nch}")
                    for cc in range(CCH):
                        ops.append(lambda hp=hp, nch=nch, cc=cc, ps=ps: nc.tensor.matmul(
                            ps,
                            lhsT=wq[cc][:, hp * 128:(hp + 1) * 128],
                            rhs=xt[cc][:, nch * 512:(nch + 1) * 512],
                            start=(cc == 0), stop=(cc == CCH - 1),
                        ))
                    ops.append(lambda hp=hp, nch=nch, ps=ps: nc.vector.tensor_copy(
                        qt[hp][:, nch * 512:(nch + 1) * 512], ps))
                for nch in range(N // 512):
                    ps = psum.tile([128, 512], f32, tag="qk", bufs=2,
                                   name=f"psk{hp}{nch}")
                    for cc in range(CCH):
                        ops.append(lambda hp=hp, nch=nch, cc=cc, ps=ps: nc.tensor.matmul(
                            ps,
                            lhsT=wq[cc][:, C + hp * 128:C + (hp + 1) * 128],
                            rhs=xt[cc][:, nch * 512:(nch + 1) * 512],
                            start=(cc == 0), stop=(cc == CCH - 1),
                        ))
                    ops.append(lambda hp=hp, nch=nch, ps=ps: nc.vector.tensor_copy(
                        kt[hp][:, nch * 512:(nch + 1) * 512], ps))
                return ops

            # proj for one 128-row output block (12 matmuls + add + dma out)
            def emit_proj(ic):
                ops = []
                pjs = []
                for (d0, dw) in ((0, 512), (512, 256)):
                    pj = psum.tile([128, 512], f32, tag="qk", bufs=2,
                                   name=f"pj{ic}_{d0}")
                    pjs.append((pj, d0, dw))
                    for cc in range(CCH):
                        ops.append(lambda ic=ic, d0=d0, dw=dw, cc=cc, pj=pj: nc.tensor.matmul(
                            pj[:, 0:dw],
                            lhsT=att[cc][:, ic * 128:(ic + 1) * 128],
                            rhs=wp[cc][:, d0:d0 + dw],
                            start=(cc == 0), stop=(cc == CCH - 1),
                        ))
                def fin(ic=ic, pjs=pjs):
                    osb = work.tile([128, C], f32, tag="osb", bufs=3,
                                    name=f"osb{ic}")
                    for (pj, d0, dw) in pjs:
                        nc.vector.tensor_add(osb[:, d0:d0 + dw], pj[:, 0:dw],
                                             bias_bc[:, d0:d0 + dw])
                    nc.sync.dma_start(out=out[ic * 128:(ic + 1) * 128, :],
                                      in_=osb)
                ops.append(fin)
                return ops

            # ---- phase 0: q/k for head pairs 0+1, prime v chunk 0 ------
            for op in emit_qk(0):
                op()
            for op in emit_qk(1):
                op()

            # v in [token, d] layout, packed [v_h(64)|1] x 12 heads + pad.
            def emit_vt(nt):
                ops = []
                vaug = vt[nt][:, 0:H * 65].rearrange("p (h e) -> p h e", e=65)
                ops.append(lambda vaug=vaug, nt=nt: nc.vector.memset(
                    vaug[:, :, 64:65], 1.0))
                ops.append(lambda nt=nt: nc.vector.memset(
                    vt[nt][:, H * 65:VW], 0.0))
                for (d0, dw) in ((0, 512), (512, 256)):
                    ps = psum.tile([128, 512], f32, tag="qk", bufs=2,
                                   name=f"psv{nt}_{d0}")
                    for cc in range(CCH):
                        ops.append(lambda nt=nt, d0=d0, dw=dw, cc=cc, ps=ps: nc.tensor.matmul(
                            ps[:, :dw],
                            lhsT=xt[cc][:, nt * 128:(nt + 1) * 128],
                            rhs=wq[cc][:, 2 * C + d0:2 * C + d0 + dw],
                            start=(cc == 0), stop=(cc == CCH - 1),
                        ))
                    ops.append(lambda vaug=vaug, d0=d0, dw=dw, ps=ps: nc.vector.tensor_copy(
                        vaug[:, d0 // 64:(d0 + dw) // 64, 0:64],
                        ps[:, :dw].rearrange("p (h e) -> p h e", e=64),
                    ))
                return ops

            for op in emit_vt(0):
                op()

            # ---- attention ---------------------------------------------
            # per-block filler: matmul-ish ops interleaved into exp-bound
            # steps so the PE never idles while ScalarE runs exp.
            NJ = N // 128                     # 16 kv chunks
            for ib in range(NQ // 512):       # 512-wide query block
                if ib == 1:
                    load.release()
                for hp in range(CCH):         # head pair
                    if ib == 0 and hp == 0:
                        filler = []
                        for nt in range(1, NJ):
                            filler.extend(emit_vt(nt))
                        per_step = 16
                    elif ib == 0 and 1 <= hp <= 4:
                        filler = emit_qk(hp + 1)
                        per_step = 3
                    elif ib == 1 and hp == 0:
                        filler = []
                        for ic in range(0, 4):
                            filler.extend(emit_proj(ic))
                        per_step = 2
                    elif ib == 1 and hp < 3:
                        per_step = 2          # keep draining proj ib0
                    else:
                        per_step = 2
                    pv = [psum.tile([128, 512], f32, tag="pv", bufs=2,
                                    name=f"pv{h2}") for h2 in range(2)]
                    for j in range(NJ):       # one kv chunk per step
                        st = psum.tile([128, 1024], f32, tag="st", bufs=2,
                                       name="st")
                        for h2 in range(2):
                            hb = h2 * 64
                            nc.tensor.matmul(
                                st[:, h2 * 512:(h2 + 1) * 512],
                                lhsT=kt[hp][hb:hb + 64, j * 128:(j + 1) * 128],
                                rhs=qt[hp][hb:hb + 64, ib * 512:(ib + 1) * 512],
                                start=True, stop=True,
                            )
                        et = work.tile([128, 1024], bf16, tag="et", bufs=4,
                                       name="et")
                        nc.scalar.activation(et, st, Exp, scale=SCALE)
                        for _ in range(per_step):
                            if filler:
                                filler.pop(0)()
                        for h2 in range(2):
                            h = hp * 2 + h2
                            nc.tensor.matmul(
                                pv[h2],
                                lhsT=vt[j][:, h * 65:h * 65 + 128],
                                rhs=et[:, h2 * 512:(h2 + 1) * 512],
                                start=(j == 0), stop=(j == NJ - 1),
                            )
                    if ib == 0:
                        # force-drain this block's remaining filler (deadline:
                        # qk(hp+1) must be ready one block ahead; vt fully
                        # consumed within block (0,0)).
                        while filler:
                            filler.pop(0)()
                    for h2 in range(2):
                        srow = work.tile([1, 512], f32, tag="srow", bufs=4,
                                         name="srow")
                        nc.vector.tensor_copy(srow, pv[h2][64:65, :])
                        sinv = work.tile([1, 512], f32, tag="sinv", bufs=4,
                                         name="sinv")
                        nc.vector.reciprocal_approx_fast(sinv, srow)
                        bc = work.tile([64, 512], f32, tag="bc", bufs=4,
                                       name="bc")
                        nc.gpsimd.partition_broadcast(bc, sinv)
                        nc.vector.tensor_mul(
                            att[hp][h2 * 64:h2 * 64 + 64, ib * 512:(ib + 1) * 512],
                            pv[h2][0:64, :],
                            bc,
                        )
            while filler:
                filler.pop(0)()
            # tail: proj for the second query block
            for ic in range(4, 8):
                for op in emit_proj(ic):
                    op()

    nc.compile()
    return nc


def _get_nc():
    if "nc" not in _NC_CACHE:
        _NC_CACHE["nc"] = _build()
    return _NC_CACHE["nc"]


def _ensure_ntff_hook():
    """The agent image's ``antenv`` lacks ``axon_hooks``; synthesize it so
    ``run_bass_kernel_spmd(trace=True)`` can capture NTFF profiles."""
    import types
    try:
        from antenv.axon_hooks import get_axon_ntff_profile_hook  # noqa: F401
        return
    except ImportError:
        pass
    import antenv
    from trn_agent_boot.trn_boot import _ntff_profile_via_ctypes
    hook = _ntff_profile_via_ctypes("/opt/axon/libaxon_pjrt.so")
    mod = types.ModuleType("antenv.axon_hooks")
    mod._hook = hook
    mod.get_axon_ntff_profile_hook = lambda: mod._hook

    def _set(h):
        mod._hook = h

    mod.set_axon_ntff_profile_hook = _set
    sys.modules["antenv.axon_hooks"] = mod
    antenv.axon_hooks = mod


def kernel(trace=False, **inputs):
    x = np.asarray(inputs["x"], np.float32)
    qkv_w = np.asarray(inputs["qkv_w"], np.float32)
    proj_w = np.asarray(inputs["proj_w"], np.float32)
    proj_b = np.asarray(inputs["proj_b"], np.float32)

    nc = _get_nc()

    xTb = np.ascontiguousarray(x.transpose(0, 2, 1)).astype(ml_dtypes.bfloat16)
    wqkvT = np.ascontiguousarray(qkv_w.T).astype(ml_dtypes.bfloat16)
    wprojT = np.ascontiguousarray(proj_w.T).astype(ml_dtypes.bfloat16)
    bias = np.ascontiguousarray(proj_b.reshape(1, C))

    in_maps = []
    for c in range(NCORES):
        b, half = divmod(c, 2)
        if half == 0:
            xTc = xTb[b]
        else:
            xTc = np.concatenate([xTb[b][:, NQ:], xTb[b][:, :NQ]], axis=1)
        in_maps.append({
            "xT": np.ascontiguousarray(xTc),
            "wqkvT": wqkvT,
            "wprojT": wprojT,
            "bias": bias,
        })

    from concourse import bass_utils
    if trace:
        _ensure_ntff_hook()
        bass_utils.upload_artifacts = lambda tmpdir: tmpdir
    res = bass_utils.run_bass_kernel_spmd(
        nc, in_maps, core_ids=list(range(NCORES)), trace=trace,
    )

    out = np.empty((B, N, C), np.float32)
    for c in range(NCORES):
        b, half = divmod(c, 2)
        out[b, half * NQ:(half + 1) * NQ, :] = res.results[c]["out"]

    if trace:
        return out, res
    return out


# revision 7
# speedup vs baseline: 1.5437x; 1.0194x over previous
"""Multi-head attention (B=4, N=2048, C=768, H=12, Dh=64) on 8 TRN2 NeuronCores.

Sharding: 2 cores per batch (data parallel on batch x sequence-split on query
rows). Each core owns 1024 query rows but computes K/V for its batch's full
2048-token sequence from host-pre-transposed activations (own half first, so
the graph is identical across cores). No collectives; the host concatenates
the 8 [1024, 768] output row-blocks.

Per-core inputs (partition dim first):
  xT     [768, 2048]  bf16  x[b].T, own-half tokens first
  wqkvT  [768, 2304]  bf16  qkv_w.T (cols 0:768 q, 768:1536 k, 1536:2304 v)
  wprojT [768, 768]   bf16  proj_w.T
  bias   [1, 768]     f32
  out    [1024, 768]  f32

v3 pipeline (from trace analysis of the 371us baseline and 342us v2):
  - score matmuls for the two heads of a pair are issued adjacently with lhsT
    base partitions 0/64 -> the PE row-tiles them concurrently.
  - scores land in a [128, 1024] PSUM tile (2 banks, h0|h1 of one kv chunk),
    double-buffered; one 1024-wide exp per chunk on ScalarE.
  - PV pair runs one step BEHIND the scores/exp so the next chunk's scores
    (and the next exp) are never queued behind a PV that waits on exp.
  - PV stationary is a 128-col window into the packed V tile
    [v_h|ones|v_{h+1}...] -> FWL weight loads; psum rows 65..127 are garbage.
  - qkv/proj matmul chains keep 2 psum tiles resident so each weight load is
    reused by two matmuls.
  - only q/k for head-pair 0 plus V chunk 0 are computed up front; V chunks
    1-15 + q/k pair 1 fill block (0,0); q/k pairs 2-5 fill blocks (0,1)-(0,4);
    ib-0 projection fills ib-1 blocks -- all interleaved into exp-bound steps.
  - normalize copies the pv psum body to SBUF immediately so the next block's
    PV accumulation is not blocked behind the reciprocal/broadcast chain.
"""

import sys

if "/opt/trn_rl_repo" not in sys.path:
    sys.path.insert(0, "/opt/trn_rl_repo")

import numpy as np
import ml_dtypes

B, N, C = 4, 2048, 768
H, Dh = 12, 64
NQ = 1024          # query rows owned by one core
SCALE = Dh ** -0.5
CCH = C // 128     # 6 contraction chunks
NCORES = 8
VW = 11 * 65 + 128  # padded width of packed v tiles (last head's 128-window)

_NC_CACHE = {}


def _build():
    import concourse.bass as bass
    import concourse.tile as tile
    import concourse.mybir as mybir
    from concourse import bacc

    f32 = mybir.dt.float32
    bf16 = mybir.dt.bfloat16
    Exp = mybir.ActivationFunctionType.Exp

    nc = bacc.Bacc(
        "TRN2",
        target_bir_lowering=False,
        debug=False,
        enable_asserts=False,
        num_devices=NCORES,
    )

    xT = nc.dram_tensor("xT", [C, N], bf16, kind="ExternalInput").ap()
    wqkvT = nc.dram_tensor("wqkvT", [C, 3 * C], bf16, kind="ExternalInput").ap()
    wprojT = nc.dram_tensor("wprojT", [C, C], bf16, kind="ExternalInput").ap()
    bias = nc.dram_tensor("bias", [1, C], f32, kind="ExternalInput").ap()
    out = nc.dram_tensor("out", [NQ, C], f32, kind="ExternalOutput").ap()

    with tile.TileContext(nc) as tc:
        from contextlib import ExitStack

        with ExitStack() as ctx:
            singles = ctx.enter_context(tc.tile_pool(name="singles", bufs=1))
            psum = ctx.enter_context(tc.tile_pool(name="psum", bufs=1, space="PSUM"))
            work = ctx.enter_context(tc.tile_pool(name="work", bufs=4))

            # ---- input DMAs --------------------------------------------
            load = tc.alloc_tile_pool(name="load", bufs=1)
            xt = [load.tile([128, N], bf16, tag=f"xt{i}", name=f"xt{i}")
                  for i in range(CCH)]
            wq = [load.tile([128, 3 * C], bf16, tag=f"wq{i}", name=f"wq{i}")
                  for i in range(CCH)]
            # xt on the sync queue; wq slices on the gpsimd queue, ordered so
            # the first matmuls can start as early as possible:
            #   q/k cols for head-pair 0, then v cols, then the rest of q/k.
            for i in range(CCH):
                nc.sync.dma_start(out=xt[i], in_=xT[i * 128:(i + 1) * 128, :])
                nc.gpsimd.dma_start(out=wq[i][:, 0:128],
                                    in_=wqkvT[i * 128:(i + 1) * 128, 0:128])
                nc.gpsimd.dma_start(out=wq[i][:, C:C + 128],
                                    in_=wqkvT[i * 128:(i + 1) * 128, C:C + 128])
            for i in range(CCH):
                nc.gpsimd.dma_start(out=wq[i][:, 2 * C:3 * C],
                                    in_=wqkvT[i * 128:(i + 1) * 128, 2 * C:3 * C])
            for i in range(CCH):
                nc.gpsimd.dma_start(out=wq[i][:, 128:C],
                                    in_=wqkvT[i * 128:(i + 1) * 128, 128:C])
                nc.gpsimd.dma_start(out=wq[i][:, C + 128:2 * C],
                                    in_=wqkvT[i * 128:(i + 1) * 128, C + 128:2 * C])
            wp = []
            for i in range(CCH):
                t = singles.tile([128, C], bf16, tag=f"wp{i}", name=f"wp{i}")
                nc.sync.dma_start(out=t, in_=wprojT[i * 128:(i + 1) * 128, :])
                wp.append(t)
            bias_bc = singles.tile([128, C], f32, tag="bias", name="bias_bc")
            nc.sync.dma_start(
                out=bias_bc,
                in_=bass.AP(tensor=bias.tensor, offset=bias.offset,
                            ap=[[0, 128]] + list(bias.ap[1:])),
            )

            # ---- qkv storage -------------------------------------------
            qt = [singles.tile([128, NQ], bf16, tag=f"qt{i}", name=f"qt{i}")
                  for i in range(CCH)]
            kt = [singles.tile([128, N], bf16, tag=f"kt{i}", name=f"kt{i}")
                  for i in range(CCH)]
            vt = [singles.tile([128, VW], bf16, tag=f"vt{i}", name=f"vt{i}")
                  for i in range(N // 128)]
            att = [singles.tile([128, NQ], bf16, tag=f"att{i}", name=f"att{i}")
                   for i in range(CCH)]

            # q/k projection for one head pair; chains kept psum-resident so
            # each wq stationary load serves two matmuls.
            def emit_qk(hp):
                ops = []
                psq = [psum.tile([128, 512], f32, tag="qk", bufs=2,
                                 name=f"psq{hp}{n}") for n in range(2)]
                for cc in range(CCH):
                    for nch in range(2):
                        ops.append(lambda hp=hp, nch=nch, cc=cc, ps=psq[nch]: nc.tensor.matmul(
                            ps,
                            lhsT=wq[cc][:, hp * 128:(hp + 1) * 128],
                            rhs=xt[cc][:, nch * 512:(nch + 1) * 512],
                            start=(cc == 0), stop=(cc == CCH - 1),
                        ))
                for nch in range(2):
                    ops.append(lambda hp=hp, nch=nch, ps=psq[nch]: nc.vector.tensor_copy(
                        qt[hp][:, nch * 512:(nch + 1) * 512], ps))
                for half in range(2):
                    psk = [psum.tile([128, 512], f32, tag="qk", bufs=2,
                                     name=f"psk{hp}{half}{n}") for n in range(2)]
                    for cc in range(CCH):
                        for i in range(2):
                            nch = half * 2 + i
                            ops.append(lambda hp=hp, nch=nch, cc=cc, ps=psk[i]: nc.tensor.matmul(
                                ps,
                                lhsT=wq[cc][:, C + hp * 128:C + (hp + 1) * 128],
                                rhs=xt[cc][:, nch * 512:(nch + 1) * 512],
                                start=(cc == 0), stop=(cc == CCH - 1),
                            ))
                    for i in range(2):
                        nch = half * 2 + i
                        ops.append(lambda hp=hp, nch=nch, ps=psk[i]: nc.vector.tensor_copy(
                            kt[hp][:, nch * 512:(nch + 1) * 512], ps))
                return ops

            # proj for one 128-row output block (12 matmuls + add + dma out)
            def emit_proj(ic):
                ops = []
                pjs = [(psum.tile([128, 512], f32, tag="qk", bufs=2,
                                  name=f"pj{ic}_{d0}"), d0, dw)
                       for (d0, dw) in ((0, 512), (512, 256))]
                for cc in range(CCH):
                    for (pj, d0, dw) in pjs:
                        ops.append(lambda ic=ic, d0=d0, dw=dw, cc=cc, pj=pj: nc.tensor.matmul(
                            pj[:, 0:dw],
                            lhsT=att[cc][:, ic * 128:(ic + 1) * 128],
                            rhs=wp[cc][:, d0:d0 + dw],
                            start=(cc == 0), stop=(cc == CCH - 1),
                        ))
                def fin(ic=ic, pjs=pjs):
                    osb = work.tile([128, C], f32, tag="osb", bufs=3,
                                    name=f"osb{ic}")
                    for (pj, d0, dw) in pjs:
                        nc.vector.tensor_add(osb[:, d0:d0 + dw], pj[:, 0:dw],
                                             bias_bc[:, d0:d0 + dw])
                    nc.sync.dma_start(out=out[ic * 128:(ic + 1) * 128, :],
                                      in_=osb)
                ops.append(fin)
                return ops

            # v in [token, d] layout, packed [v_h(64)|1] x 12 heads + pad.
            def emit_vt(nt):
                ops = []
                vaug = vt[nt][:, 0:H * 65].rearrange("p (h e) -> p h e", e=65)
                ops.append(lambda vaug=vaug: nc.vector.memset(
                    vaug[:, :, 64:65], 1.0))
                ops.append(lambda nt=nt: nc.vector.memset(
                    vt[nt][:, H * 65:VW], 0.0))
                pss = [(psum.tile([128, 512], f32, tag="qk", bufs=2,
                                  name=f"psv{nt}_{d0}"), d0, dw)
                       for (d0, dw) in ((0, 512), (512, 256))]
                for cc in range(CCH):
                    for (ps, d0, dw) in pss:
                        ops.append(lambda nt=nt, d0=d0, dw=dw, cc=cc, ps=ps: nc.tensor.matmul(
                            ps[:, :dw],
                            lhsT=xt[cc][:, nt * 128:(nt + 1) * 128],
                            rhs=wq[cc][:, 2 * C + d0:2 * C + d0 + dw],
                            start=(cc == 0), stop=(cc == CCH - 1),
                        ))
                for (ps, d0, dw) in pss:
                    ops.append(lambda vaug=vaug, d0=d0, dw=dw, ps=ps: nc.vector.tensor_copy(
                        vaug[:, d0 // 64:(d0 + dw) // 64, 0:64],
                        ps[:, :dw].rearrange("p (h e) -> p h e", e=64),
                    ))
                return ops

            # ---- phase 0: q/k for head pair 0, prime v chunk 0 ---------
            for op in emit_qk(0):
                op()
            for op in emit_vt(0):
                op()

            # ---- attention ---------------------------------------------
            # per-block filler: matmul-ish ops interleaved into exp-bound
            # steps so the PE never idles while ScalarE runs exp.
            NJ = N // 128                     # 16 kv chunks
            filler = []
            for ib in range(NQ // 512):       # 512-wide query block
                if ib == 1:
                    load.release()
                for hp in range(CCH):         # head pair
                    if ib == 0 and hp == 0:
                        filler = []
                        for nt in range(1, NJ):
                            filler.extend(emit_vt(nt))
                        filler.extend(emit_qk(1))
                        per_step = 18
                    elif ib == 0 and 1 <= hp <= 4:
                        filler = emit_qk(hp + 1)
                        per_step = 3
                    elif ib == 1 and hp == 0:
                        filler = []
                        for ic in range(0, 4):
                            filler.extend(emit_proj(ic))
                        per_step = 2
                    else:
                        per_step = 2          # drain leftovers
                    pv = [psum.tile([128, 512], f32, tag="pv", bufs=2,
                                    name=f"pv{h2}") for h2 in range(2)]
                    prev_pv = None
                    for j in range(NJ):       # one kv chunk per step
                        st = psum.tile([128, 1024], f32, tag="st", bufs=2,
                                       name="st")
                        for h2 in range(2):
                            hb = h2 * 64
                            nc.tensor.matmul(
                                st[:, h2 * 512:(h2 + 1) * 512],
                                lhsT=kt[hp][hb:hb + 64, j * 128:(j + 1) * 128],
                                rhs=qt[hp][hb:hb + 64, ib * 512:(ib + 1) * 512],
                                start=True, stop=True,
                            )
                        et = work.tile([128, 1024], bf16, tag="et", bufs=4,
                                       name="et")
                        nc.scalar.activation(et, st, Exp, scale=SCALE)
                        for _ in range(per_step):
                            if filler:
                                filler.pop(0)()
                        if prev_pv is not None:
                            prev_pv()

                        def mk_pv(j=j, et=et, pv=pv, hp=hp):
                            for h2 in range(2):
                                h = hp * 2 + h2
                                nc.tensor.matmul(
                                    pv[h2],
                                    lhsT=vt[j][:, h * 65:h * 65 + 128],
                                    rhs=et[:, h2 * 512:(h2 + 1) * 512],
                                    start=(j == 0), stop=(j == NJ - 1),
                                )
                        prev_pv = mk_pv
                    prev_pv()
                    if ib == 0:
                        # force-drain: qk(hp+1) must be ready one block ahead;
                        # vt fully consumed within block (0,0).
                        while filler:
                            filler.pop(0)()
                    for h2 in range(2):
                        srow = work.tile([1, 512], f32, tag="srow", bufs=4,
                                         name="srow")
                        nc.vector.tensor_copy(srow, pv[h2][64:65, :])
                        pvb = work.tile([64, 512], f32, tag="pvb", bufs=4,
                                        name="pvb")
                        nc.vector.tensor_copy(pvb, pv[h2][0:64, :])
                        sinv = work.tile([1, 512], f32, tag="sinv", bufs=4,
                                         name="sinv")
                        nc.vector.reciprocal_approx_fast(sinv, srow)
                        bc = work.tile([64, 512], f32, tag="bc", bufs=4,
                                       name="bc")
                        nc.gpsimd.partition_broadcast(bc, sinv)
                        nc.vector.tensor_mul(
                            att[hp][h2 * 64:h2 * 64 + 64, ib * 512:(ib + 1) * 512],
                            pvb,
                            bc,
                        )
            while filler:
                filler.pop(0)()
            # tail: proj for the second query block
            for ic in range(4, 8):
                for op in emit_proj(ic):
                    op()

    nc.compile()
    return nc


def _get_nc():
    if "nc" not in _NC_CACHE:
        _NC_CACHE["nc"] = _build()
    return _NC_CACHE["nc"]


def _ensure_ntff_hook():
    """The agent image's ``antenv`` lacks ``axon_hooks``; synthesize it so
    ``run_bass_kernel_spmd(trace=True)`` can capture NTFF profiles."""
    import types
    try:
        from antenv.axon_hooks import get_axon_ntff_profile_hook  # noqa: F401
        return
    except ImportError:
        pass
    import antenv
    from trn_agent_boot.trn_boot import _ntff_profile_via_ctypes
    hook = _ntff_profile_via_ctypes("/opt/axon/libaxon_pjrt.so")
    mod = types.ModuleType("antenv.axon_hooks")
    mod._hook = hook
    mod.get_axon_ntff_profile_hook = lambda: mod._hook

    def _set(h):
        mod._hook = h

    mod.set_axon_ntff_profile_hook = _set
    sys.modules["antenv.axon_hooks"] = mod
    antenv.axon_hooks = mod


def kernel(trace=False, **inputs):
    x = np.asarray(inputs["x"], np.float32)
    qkv_w = np.asarray(inputs["qkv_w"], np.float32)
    proj_w = np.asarray(inputs["proj_w"], np.float32)
    proj_b = np.asarray(inputs["proj_b"], np.float32)

    nc = _get_nc()

    xTb = np.ascontiguousarray(x.transpose(0, 2, 1)).astype(ml_dtypes.bfloat16)
    wqkvT = np.ascontiguousarray(qkv_w.T).astype(ml_dtypes.bfloat16)
    wprojT = np.ascontiguousarray(proj_w.T).astype(ml_dtypes.bfloat16)
    bias = np.ascontiguousarray(proj_b.reshape(1, C))

    in_maps = []
    for c in range(NCORES):
        b, half = divmod(c, 2)
        if half == 0:
            xTc = xTb[b]
        else:
            xTc = np.concatenate([xTb[b][:, NQ:], xTb[b][:, :NQ]], axis=1)
        in_maps.append({
            "xT": np.ascontiguousarray(xTc),
            "wqkvT": wqkvT,
            "wprojT": wprojT,
            "bias": bias,
        })

    from concourse import bass_utils
    if trace:
        _ensure_ntff_hook()
        bass_utils.upload_artifacts = lambda tmpdir: tmpdir
    res = bass_utils.run_bass_kernel_spmd(
        nc, in_maps, core_ids=list(range(NCORES)), trace=trace,
    )

    out = np.empty((B, N, C), np.float32)
    for c in range(NCORES):
        b, half = divmod(c, 2)
        out[b, half * NQ:(half + 1) * NQ, :] = res.results[c]["out"]

    if trace:
        return out, res
    return out


# revision 8
# speedup vs baseline: 1.6440x; 1.0650x over previous
"""Multi-head attention (B=4, N=2048, C=768, H=12, Dh=64) on 8 TRN2 NeuronCores.

Sharding (v4): head-parallel within each batch. Core (b, g) (g = core % 2)
computes q/k/v for heads 6g..6g+5 of batch b over the FULL 2048-token
sequence -- no duplicated K/V work between the pair -- runs attention for its
6 heads x 2048 queries, and produces a PARTIAL projection over its 384
channels. The host sums the two partial [2048, 768] outputs per batch
(host-side all-reduce; no device collectives). The bias rides in core g=0's
input; core g=1 receives zeros, keeping the program SPMD-identical.

Per-core inputs (partition dim first):
  xT     [768, 2048]  bf16  x[b].T (same for both cores of a pair)
  wqkvT  [768, 1152]  bf16  columns [q_g | k_g | v_g], 384 each, pre-sliced
  wprojT [384, 768]   bf16  proj_w.T rows for this core's 384 channels
  bias   [1, 768]     f32   real bias for g=0, zeros for g=1
  out    [2048, 768]  f32   partial projection

Pipeline (from trace analysis of the 371us baseline / 342us v2 / 335us v3):
  - scores for the two heads of a pair go to a [128, 1024] PSUM tile
    (2 banks), double-buffered; one 1024-wide exp per kv chunk on ScalarE.
    ScalarE (exp) is the saturated engine in steady state (~209us).
  - PV pair runs one step behind the scores/exp so the next exp is never
    blocked behind a PV waiting on the current one.
  - PV stationary is a 128-col window into the packed V tile
    [v_h|ones|v_{h+1}...] -> psum row 64 is the softmax denominator, rows
    65..127 garbage.
  - qkv/proj matmul chains keep 2 psum tiles resident so each weight load is
    reused by two matmuls.
  - q/k for head-pair 0 plus V chunk 0 run up front; V chunks 1-15 + q/k for
    pair 1 fill block (0,0); q/k pair 2 fills (0,1); per-ib projection fills
    the next ib's first block -- all interleaved into exp-bound steps.
  - normalize copies the pv psum body to SBUF immediately so the next block's
    PV accumulation is not blocked behind the reciprocal/broadcast chain.
"""

import sys

if "/opt/trn_rl_repo" not in sys.path:
    sys.path.insert(0, "/opt/trn_rl_repo")

import numpy as np
import ml_dtypes

B, N, C = 4, 2048, 768
H, Dh = 12, 64
HL = 6             # heads per core
CL = HL * Dh       # 384 local channels
SCALE = Dh ** -0.5
CCH = C // 128     # 6 contraction chunks (x/qkv input dim)
CCL = CL // 128    # 3 local head-pair chunks
NCORES = 8
VW = (HL - 1) * 65 + 128  # padded width of packed v tiles (453)

_NC_CACHE = {}


def _build():
    import concourse.bass as bass
    import concourse.tile as tile
    import concourse.mybir as mybir
    from concourse import bacc

    f32 = mybir.dt.float32
    bf16 = mybir.dt.bfloat16
    Exp = mybir.ActivationFunctionType.Exp

    nc = bacc.Bacc(
        "TRN2",
        target_bir_lowering=False,
        debug=False,
        enable_asserts=False,
        num_devices=NCORES,
    )

    xT = nc.dram_tensor("xT", [C, N], bf16, kind="ExternalInput").ap()
    wqkvT = nc.dram_tensor("wqkvT", [C, 3 * CL], bf16, kind="ExternalInput").ap()
    wprojT = nc.dram_tensor("wprojT", [CL, C], bf16, kind="ExternalInput").ap()
    bias = nc.dram_tensor("bias", [1, C], f32, kind="ExternalInput").ap()
    out = nc.dram_tensor("out", [N, C], f32, kind="ExternalOutput").ap()

    with tile.TileContext(nc) as tc:
        from contextlib import ExitStack

        with ExitStack() as ctx:
            singles = ctx.enter_context(tc.tile_pool(name="singles", bufs=1))
            psum = ctx.enter_context(tc.tile_pool(name="psum", bufs=1, space="PSUM"))
            work = ctx.enter_context(tc.tile_pool(name="work", bufs=4))

            # ---- input DMAs --------------------------------------------
            load = tc.alloc_tile_pool(name="load", bufs=1)
            xt = [load.tile([128, N], bf16, tag=f"xt{i}", name=f"xt{i}")
                  for i in range(CCH)]
            wq = [load.tile([128, 3 * CL], bf16, tag=f"wq{i}", name=f"wq{i}")
                  for i in range(CCH)]
            # xt on the sync queue; wq slices on the gpsimd queue ordered so
            # the first matmuls can start早: q/k pair-0 cols, v cols, rest.
            for i in range(CCH):
                nc.sync.dma_start(out=xt[i], in_=xT[i * 128:(i + 1) * 128, :])
                nc.gpsimd.dma_start(out=wq[i][:, 0:128],
                                    in_=wqkvT[i * 128:(i + 1) * 128, 0:128])
                nc.gpsimd.dma_start(out=wq[i][:, CL:CL + 128],
                                    in_=wqkvT[i * 128:(i + 1) * 128, CL:CL + 128])
            for i in range(CCH):
                nc.gpsimd.dma_start(out=wq[i][:, 2 * CL:3 * CL],
                                    in_=wqkvT[i * 128:(i + 1) * 128, 2 * CL:3 * CL])
            for i in range(CCH):
                nc.gpsimd.dma_start(out=wq[i][:, 128:CL],
                                    in_=wqkvT[i * 128:(i + 1) * 128, 128:CL])
                nc.gpsimd.dma_start(out=wq[i][:, CL + 128:2 * CL],
                                    in_=wqkvT[i * 128:(i + 1) * 128, CL + 128:2 * CL])
            wp = []
            for i in range(CCL):
                t = singles.tile([128, C], bf16, tag=f"wp{i}", name=f"wp{i}")
                nc.sync.dma_start(out=t, in_=wprojT[i * 128:(i + 1) * 128, :])
                wp.append(t)
            bias_bc = singles.tile([128, C], f32, tag="bias", name="bias_bc")
            nc.sync.dma_start(
                out=bias_bc,
                in_=bass.AP(tensor=bias.tensor, offset=bias.offset,
                            ap=[[0, 128]] + list(bias.ap[1:])),
            )

            # ---- qkv storage -------------------------------------------
            qt = [singles.tile([128, N], bf16, tag=f"qt{i}", name=f"qt{i}")
                  for i in range(CCL)]
            kt = [singles.tile([128, N], bf16, tag=f"kt{i}", name=f"kt{i}")
                  for i in range(CCL)]
            vt = [singles.tile([128, VW], bf16, tag=f"vt{i}", name=f"vt{i}")
                  for i in range(N // 128)]
            att = [singles.tile([128, N], bf16, tag=f"att{i}", name=f"att{i}")
                   for i in range(CCL)]

            # q/k projection for one local head pair; chains psum-resident in
            # pairs so each wq stationary load serves two matmuls.
            def emit_qk(hp):
                ops = []
                for (dst, base) in ((qt, 0), (kt, CL)):
                    for half in range(2):
                        pss = [psum.tile([128, 512], f32, tag="qk", bufs=2,
                                         name=f"qk{hp}{base}{half}{n}")
                               for n in range(2)]
                        for cc in range(CCH):
                            for i in range(2):
                                nch = half * 2 + i
                                ops.append(lambda hp=hp, base=base, nch=nch, cc=cc, ps=pss[i]: nc.tensor.matmul(
                                    ps,
                                    lhsT=wq[cc][:, base + hp * 128:base + (hp + 1) * 128],
                                    rhs=xt[cc][:, nch * 512:(nch + 1) * 512],
                                    start=(cc == 0), stop=(cc == CCH - 1),
                                ))
                        for i in range(2):
                            nch = half * 2 + i
                            ops.append(lambda dst=dst, hp=hp, nch=nch, ps=pss[i]: nc.vector.tensor_copy(
                                dst[hp][:, nch * 512:(nch + 1) * 512], ps))
                return ops

            # partial proj for one 128-row output block (6 matmuls + add + dma)
            def emit_proj(ic):
                ops = []
                pjs = [(psum.tile([128, 512], f32, tag="qk", bufs=2,
                                  name=f"pj{ic}_{d0}"), d0, dw)
                       for (d0, dw) in ((0, 512), (512, 256))]
                for cc in range(CCL):
                    for (pj, d0, dw) in pjs:
                        ops.append(lambda ic=ic, d0=d0, dw=dw, cc=cc, pj=pj: nc.tensor.matmul(
                            pj[:, 0:dw],
                            lhsT=att[cc][:, ic * 128:(ic + 1) * 128],
                            rhs=wp[cc][:, d0:d0 + dw],
                            start=(cc == 0), stop=(cc == CCL - 1),
                        ))
                def fin(ic=ic, pjs=pjs):
                    osb = work.tile([128, C], f32, tag="osb", bufs=3,
                                    name=f"osb{ic}")
                    for (pj, d0, dw) in pjs:
                        nc.vector.tensor_add(osb[:, d0:d0 + dw], pj[:, 0:dw],
                                             bias_bc[:, d0:d0 + dw])
                    nc.sync.dma_start(out=out[ic * 128:(ic + 1) * 128, :],
                                      in_=osb)
                ops.append(fin)
                return ops

            # v in [token, d] layout, packed [v_h(64)|1] x 6 heads + pad.
            def emit_vt(nt):
                ops = []
                vaug = vt[nt][:, 0:HL * 65].rearrange("p (h e) -> p h e", e=65)
                ops.append(lambda vaug=vaug: nc.vector.memset(
                    vaug[:, :, 64:65], 1.0))
                ops.append(lambda nt=nt: nc.vector.memset(
                    vt[nt][:, HL * 65:VW], 0.0))
                ps = psum.tile([128, 512], f32, tag="qk", bufs=2,
                               name=f"psv{nt}")
                for cc in range(CCH):
                    ops.append(lambda nt=nt, cc=cc, ps=ps: nc.tensor.matmul(
                        ps[:, 0:CL],
                        lhsT=xt[cc][:, nt * 128:(nt + 1) * 128],
                        rhs=wq[cc][:, 2 * CL:3 * CL],
                        start=(cc == 0), stop=(cc == CCH - 1),
                    ))
                ops.append(lambda vaug=vaug, ps=ps: nc.vector.tensor_copy(
                    vaug[:, :, 0:64],
                    ps[:, 0:CL].rearrange("p (h e) -> p h e", e=64),
                ))
                return ops

            # ---- phase 0: q/k for head pair 0, prime v chunk 0 ---------
            for op in emit_qk(0):
                op()
            for op in emit_vt(0):
                op()

            # ---- attention ---------------------------------------------
            # per-block filler: matmul-ish ops interleaved into exp-bound
            # steps so the PE never idles while ScalarE runs exp.
            NJ = N // 128                     # 16 kv chunks
            filler = []
            for ib in range(N // 512):        # 512-wide query block
                if ib == 1:
                    load.release()
                for hp in range(CCL):         # local head pair
                    if ib == 0 and hp == 0:
                        filler = []
                        for nt in range(1, NJ):
                            filler.extend(emit_vt(nt))
                        filler.extend(emit_qk(1))
                        per_step = 12
                    elif ib == 0 and hp == 1:
                        filler = emit_qk(2)
                        per_step = 4
                    elif ib >= 1 and hp == 0:
                        filler = []
                        for ic in range((ib - 1) * 4, ib * 4):
                            filler.extend(emit_proj(ic))
                        per_step = 2
                    else:
                        per_step = 2          # drain leftovers
                    pv = [psum.tile([128, 512], f32, tag="pv", bufs=2,
                                    name=f"pv{h2}") for h2 in range(2)]
                    prev_pv = None
                    for j in range(NJ):       # one kv chunk per step
                        st = psum.tile([128, 1024], f32, tag="st", bufs=2,
                                       name="st")
                        for h2 in range(2):
                            hb = h2 * 64
                            nc.tensor.matmul(
                                st[:, h2 * 512:(h2 + 1) * 512],
                                lhsT=kt[hp][hb:hb + 64, j * 128:(j + 1) * 128],
                                rhs=qt[hp][hb:hb + 64, ib * 512:(ib + 1) * 512],
                                start=True, stop=True,
                            )
                        et = work.tile([128, 1024], bf16, tag="et", bufs=4,
                                       name="et")
                        nc.scalar.activation(et, st, Exp, scale=SCALE)
                        for _ in range(per_step):
                            if filler:
                                filler.pop(0)()
                        if prev_pv is not None:
                            prev_pv()

                        def mk_pv(j=j, et=et, pv=pv, hp=hp):
                            for h2 in range(2):
                                h = hp * 2 + h2
                                nc.tensor.matmul(
                                    pv[h2],
                                    lhsT=vt[j][:, h * 65:h * 65 + 128],
                                    rhs=et[:, h2 * 512:(h2 + 1) * 512],
                                    start=(j == 0), stop=(j == NJ - 1),
                                )
                        prev_pv = mk_pv
                    prev_pv()
                    if ib == 0:
                        # force-drain: qk(hp+1) must be ready one block ahead;
                        # vt fully consumed within block (0,0).
                        while filler:
                            filler.pop(0)()
                    for h2 in range(2):
                        srow = work.tile([1, 512], f32, tag="srow", bufs=4,
                                         name="srow")
                        nc.vector.tensor_copy(srow, pv[h2][64:65, :])
                        pvb = work.tile([64, 512], f32, tag="pvb", bufs=4,
                                        name="pvb")
                        nc.vector.tensor_copy(pvb, pv[h2][0:64, :])
                        sinv = work.tile([1, 512], f32, tag="sinv", bufs=4,
                                         name="sinv")
                        nc.vector.reciprocal_approx_fast(sinv, srow)
                        bc = work.tile([64, 512], f32, tag="bc", bufs=4,
                                       name="bc")
                        nc.gpsimd.partition_broadcast(bc, sinv)
                        nc.vector.tensor_mul(
                            att[hp][h2 * 64:h2 * 64 + 64, ib * 512:(ib + 1) * 512],
                            pvb,
                            bc,
                        )
            while filler:
                filler.pop(0)()
            # tail: proj for the last query block
            for ic in range(12, 16):
                for op in emit_proj(ic):
                    op()

    nc.compile()
    return nc


def _get_nc():
    if "nc" not in _NC_CACHE:
        _NC_CACHE["nc"] = _build()
    return _NC_CACHE["nc"]


def _ensure_ntff_hook():
    """The agent image's ``antenv`` lacks ``axon_hooks``; synthesize it so
    ``run_bass_kernel_spmd(trace=True)`` can capture NTFF profiles."""
    import types
    try:
        from antenv.axon_hooks import get_axon_ntff_profile_hook  # noqa: F401
        return
    except ImportError:
        pass
    import antenv
    from trn_agent_boot.trn_boot import _ntff_profile_via_ctypes
    hook = _ntff_profile_via_ctypes("/opt/axon/libaxon_pjrt.so")
    mod = types.ModuleType("antenv.axon_hooks")
    mod._hook = hook
    mod.get_axon_ntff_profile_hook = lambda: mod._hook

    def _set(h):
        mod._hook = h

    mod.set_axon_ntff_profile_hook = _set
    sys.modules["antenv.axon_hooks"] = mod
    antenv.axon_hooks = mod


def kernel(trace=False, **inputs):
    x = np.asarray(inputs["x"], np.float32)
    qkv_w = np.asarray(inputs["qkv_w"], np.float32)
    proj_w = np.asarray(inputs["proj_w"], np.float32)
    proj_b = np.asarray(inputs["proj_b"], np.float32)

    nc = _get_nc()

    xTb = np.ascontiguousarray(x.transpose(0, 2, 1)).astype(ml_dtypes.bfloat16)
    wqkvT = np.ascontiguousarray(qkv_w.T).astype(ml_dtypes.bfloat16)  # [768, 2304]
    wprojT = np.ascontiguousarray(proj_w.T).astype(ml_dtypes.bfloat16)  # [768, 768]
    bias = np.ascontiguousarray(proj_b.reshape(1, C)).astype(np.float32)
    zbias = np.zeros_like(bias)

    in_maps = []
    for c in range(NCORES):
        b, g = divmod(c, 2)
        cols = slice(g * CL, (g + 1) * CL)
        wq_loc = np.concatenate(
            [wqkvT[:, 0:C][:, cols], wqkvT[:, C:2 * C][:, cols],
             wqkvT[:, 2 * C:3 * C][:, cols]], axis=1)
        in_maps.append({
            "xT": xTb[b],
            "wqkvT": np.ascontiguousarray(wq_loc),
            "wprojT": np.ascontiguousarray(wprojT[g * CL:(g + 1) * CL, :]),
            "bias": bias if g == 0 else zbias,
        })

    from concourse import bass_utils
    if trace:
        _ensure_ntff_hook()
        bass_utils.upload_artifacts = lambda tmpdir: tmpdir
    res = bass_utils.run_bass_kernel_spmd(
        nc, in_maps, core_ids=list(range(NCORES)), trace=trace,
    )

    out = np.empty((B, N, C), np.float32)
    for b in range(B):
        out[b] = res.results[2 * b]["out"]
        out[b] += res.results[2 * b + 1]["out"]

    if trace:
        return out, res
    return out
